# revision 27
# baseline (speedup 1.0000x reference)
"""Channel-attention (CAM) Trainium2 Bass kernel.

Reference computation (per batch n):
    v = x[n].reshape(C, S)                 # C=512, S=H*W=4096
    energy = v @ v.T                       # (C, C)
    att = softmax(max_row(energy) - energy, axis=-1)
        = exp(min_row(energy) - energy) / Z
    out[n] = gamma * (att @ v) + x[n]

Sharding: data-parallel over N=8 batches across 8 NeuronCores; each core
computes one full C x C attention locally (no collectives).

The kernel is HBM-bound: 8 MiB in + 8 MiB out at the 360 GB/s modeled DMA
rate is ~46.6 us; every energy element needs every input byte, so the
output stream cannot start until all loads land plus one softmax-pipeline
latency. The design minimizes that mid-kernel DMA bubble:

  1. Loads stream as 512-col stripes. The LAST stripe (cols 3584-4096) is
     issued from the ACT queue so it lands EARLY in the DMA order; the
     true last-arriving stripe (6) then flows through the already-warm
     bulk pipeline instead of a cold end-of-load convoy.
  2. Per stripe, DVE casts to bf16 (vb) and GpSimd to fp8-e4m3 (vb8).
  3. vT: every chunk is PE-transposed (bf16) during the load phase into
     rotating PSUM staging; PSUM->SBUF copies go 3/4 ACT / 1/4 DVE so the
     DVE queue is empty when the final stripe lands (the copy stream is
     the end-game serial bottleneck); the last four chunks alternate
     engines.
  4. einsum1 energy = sum_k vT_k.T @ vT_k (bf16, f32 PSUM): UPPER
     BLOCK-TRIANGLE only (row ci keeps cols >= ci*128); lower blocks are
     rebuilt by f32 PE transposes of closed upper blocks. Row 0 (the
     first-stored row) is issued first over all chunks in arrival order;
     rows 1-3 fill PE idle slots and may spill past loads-end.
  5. Row pipeline: DVE row-min -> ACT exp (accum_out = Z) -> PE
     transposes of P (copies alternate ACT/DVE, cast to fp8) -> einsum2
     att @ v as fp8 DoubleRow pair-matmuls -> fused epilogue
     xf = o_ps*(gamma/Z) + xf in ONE DVE scalar_tensor_tensor (first 3
     column-groups per row; later groups alternate with an ACT-scale +
     GpSimd-add path) -> store straight from xf. Two einsum2 groups per
     row are deferred into the next row''s softmax window.

Cost-model (TimelineSim): ~60.8 us/core vs ~47 us DMA-roofline; the
remaining gap is the post-load softmax-pipeline latency (~10 us bubble:
900ns DMA-completion sem, last-stripe cast/transpose/copy drain, row-0
close, reduce+exp+PT+einsum2+epilogue chain, ~1.3 us store-issue
overhead).
"""

import os
from contextlib import ExitStack

import numpy as np

import concourse.bass as bass
import concourse.tile as tile
from concourse import bacc, mybir
from concourse.bass_utils import run_bass_kernel_spmd
from concourse.masks import make_identity

N_CORES = 8
C = 512
S = 4096
P = 128
CI = C // P  # 4 c-chunks
KD = S // P  # 32 s-chunks of 128
SJW = 512
SJ = S // SJW  # 8 s-chunks of 512
TGROUPS = 1  # xbar-transpose DMA granularity (groups over the tail of S)

F32 = mybir.dt.float32
BF16 = mybir.dt.bfloat16
FP8 = mybir.dt.float8e4


def _body(ctx: ExitStack, tc: tile.TileContext, out: bass.AP, x: bass.AP,
          gamma: bass.AP):
    nc = tc.nc

    persist = ctx.enter_context(tc.tile_pool(name="persist", bufs=1))
    xf = persist.tile([P, CI, S], F32, name="xf")
    vb = persist.tile([P, CI, S], BF16, name="vb")
    vb8 = persist.tile([P, CI, S], FP8, name="vb8")
    vbT = persist.tile([P, KD, C], BF16, name="vbT")
    p_sb = persist.tile([P, CI, C], BF16, name="p_sb")
    pt_sb = persist.tile([P, CI, C], FP8, name="pt_sb")
    ident = persist.tile([P, P], BF16, name="ident")
    identf = persist.tile([P, P], F32, name="identf")
    gamma_sb = persist.tile([P, 1], F32, name="gamma_sb")
    mn = persist.tile([P, CI], F32, name="mn")
    zsum = persist.tile([P, CI], F32, name="zsum")
    msc = persist.tile([P, CI], F32, name="msc")

    x3 = x.rearrange("(ci p) s -> p ci s", p=P)
    out3 = out.rearrange("(ci p) s -> p ci s", p=P)

    # ---- loads first (earliest possible DMA start), then casts ----
    # The LAST stripe (cols 3584-4096) is issued from the ACT/DVE queues so
    # it lands EARLY in the real DMA order (their HWDGE slots race ahead of
    # the 28 sync-queued loads). The compile-time tile scheduler then emits
    # every stripe-7-dependent instruction early in each engine's in-order
    # stream, and the true last-arriving stripe (6) flows through the
    # already-warm bulk pipeline -- this removes the end-of-load convoy
    # that otherwise delays the first energy row by ~3us.
    KPE = 28                    # chunks in stripes 0-6 (stripe 6 arrives last)
    stripe_cols = [512] * 8
    tail_sl = slice(KPE * P, KD * P)
    for ci in range(CI):
        nc.scalar.dma_start(out=xf[:, ci, tail_sl], in_=x3[:, ci, tail_sl])
    col = 0
    for w in stripe_cols[:-1]:
        sl = slice(col, col + w)
        for ci in range(CI):
            nc.sync.dma_start(out=xf[:, ci, sl], in_=x3[:, ci, sl])
        col += w

    make_identity(nc, ident)
    make_identity(nc, identf)
    nc.gpsimd.dma_start(out=gamma_sb[:, :], in_=gamma.to_broadcast((P, 1)))

    # casts in arrival order: stripe 7 first (it lands first), then 0-6;
    # DVE makes the bf16 copy, GpSimd the fp8 copy for einsum2
    for ci in range(CI):
        nc.vector.tensor_copy(out=vb[:, ci, tail_sl], in_=xf[:, ci, tail_sl])
        nc.gpsimd.tensor_copy(out=vb8[:, ci, tail_sl], in_=xf[:, ci, tail_sl])
    col = 0
    for w in stripe_cols[:-1]:
        sl = slice(col, col + w)
        for ci in range(CI):
            nc.vector.tensor_copy(out=vb[:, ci, sl], in_=xf[:, ci, sl])
            nc.gpsimd.tensor_copy(out=vb8[:, ci, sl], in_=xf[:, ci, sl])
        col += w

    # ---- vT: ALL chunks transposed on the PE (it is idle during loads;
    # xbar transposes would serialize ~3.4us of HWDGE overhead right on the
    # load->softmax critical path) ----
    # PSUM->SBUF copies alternate ACT/DVE: a single-engine copy stream
    # (28 x 612ns on ACT) would be the serial bottleneck gating einsum1
    # PSUM pools are bank-granular: 4 energy banks + 4 rotating banks
    opool = ctx.enter_context(tc.tile_pool(name="opool", bufs=4, space="PSUM"))
    KORDER = list(range(KPE, KD)) + list(range(KPE))  # chunk arrival order
    from contextlib import nullcontext
    for i, k in enumerate(KORDER):
        tp_ps = opool.tile([P, C], BF16, name="tp_ps", tag="op")
        # last-arriving chunks: high priority so the PE prefers their
        # transposes over rows-1-3 einsum work the moment casts land
        hp = tc.high_priority() if i >= KD - 4 else nullcontext()
        with hp:
            for ci in range(CI):
                nc.tensor.transpose(
                    out=tp_ps[:, ci * P:(ci + 1) * P],
                    in_=vb[:, ci, k * P:(k + 1) * P],
                    identity=ident[:, :],
                )
        # ACT takes 3/4 of the copies so DVE (which also does every cast)
        # keeps an empty queue near the end of the load phase; the last
        # stripe alternates so its four copies drain on two engines
        if i >= KD - 4:
            use_dve = (i % 2 == 1)
        else:
            use_dve = (i % 4 == 1)
        if use_dve:
            nc.vector.tensor_copy(out=vbT[:, k, :], in_=tp_ps[:, :])
        else:
            nc.scalar.copy(out=vbT[:, k, :], in_=tp_ps[:, :])

    # ---- einsum1: energy = v @ v.T ----
    # Row-block 0 first over ALL chunks (it gates the first store); rows 1-3
    # follow with higher scheduler priority numbers, so the PE prefers
    # row-0-critical work whenever both are ready.
    epool = ctx.enter_context(tc.tile_pool(name="epool", bufs=4, space="PSUM"))
    e_ps = [epool.tile([P, C], F32, name=f"e{ci}", tag="et") for ci in range(CI)]
    # energy is symmetric: compute only the upper block-triangle (row ci
    # keeps columns >= ci*P); the lower blocks are reconstructed by
    # transposing the finished upper blocks on the PE (f32)
    for i, k in enumerate(KORDER):
        nc.tensor.matmul(
            e_ps[0][:, :],
            lhsT=vbT[:, k, 0:P],
            rhs=vbT[:, k, :],
            start=(i == 0),
            stop=(i == KD - 1),
        )

    otmp_pool = ctx.enter_context(tc.tile_pool(name="otmp", bufs=6))

    def einsum2_group(ci, sj, _unused):
        # fp8 DoubleRow: each matmul contracts a PAIR of 128-d chunks
        o_ps = opool.tile([P, SJW], F32, name="o_ps", tag="op")
        for h in range(CI // 2):
            nc.tensor.matmul(
                o_ps[:, :],
                lhsT=pt_sb[:, 2 * h:2 * h + 2, ci * P:(ci + 1) * P],
                rhs=vb8[:, 2 * h:2 * h + 2, sj * SJW:(sj + 1) * SJW],
                start=(h == 0),
                stop=(h == CI // 2 - 1),
                perf_mode=mybir.MatmulPerfMode.DoubleRow,
            )
        hs = slice(sj * SJW, (sj + 1) * SJW)
        # first three groups of a row all take the short DVE path so the
        # store stream ramps without a gap; afterwards alternate DVE /
        # ACT+GpSimd to balance engine load
        if sj < 3 or sj % 2 == 0:
            # fused epilogue: xf = o_ps * (gamma/Z) + xf in ONE DVE op
            nc.vector.scalar_tensor_tensor(
                out=xf[:, ci, hs], in0=o_ps[:, :], scalar=msc[:, ci:ci + 1],
                in1=xf[:, ci, hs], op0=mybir.AluOpType.mult,
                op1=mybir.AluOpType.add,
            )
        else:
            # ACT scale-copy PSUM->SBUF, then GpSimd (no PSUM access) adds x
            o_tmp = otmp_pool.tile([P, SJW], F32, name="o_tmp", tag="ot")
            nc.scalar.activation(
                out=o_tmp[:, :], in_=o_ps[:, :],
                func=mybir.ActivationFunctionType.Copy,
                scale=msc[:, ci:ci + 1], bias=0.0,
            )
            nc.gpsimd.tensor_add(out=xf[:, ci, hs], in0=o_tmp[:, :],
                                 in1=xf[:, ci, hs])
        nc.sync.dma_start(out=out3[:, ci, hs], in_=xf[:, ci, hs])

    def softmax_pt(ci):
        # softmax (reversed): P = exp(min_row(e) - e), Z = row sums
        nc.vector.tensor_reduce(
            out=mn[:, ci:ci + 1], in_=e_ps[ci][:, :],
            axis=mybir.AxisListType.X, op=mybir.AluOpType.min,
        )
        nc.scalar.activation(
            out=p_sb[:, ci, :], in_=e_ps[ci][:, :],
            func=mybir.ActivationFunctionType.Exp,
            bias=mn[:, ci:ci + 1], scale=-1.0,
            accum_out=zsum[:, ci:ci + 1],
        )
        nc.vector.reciprocal(out=msc[:, ci:ci + 1], in_=zsum[:, ci:ci + 1])
        nc.vector.tensor_mul(out=msc[:, ci:ci + 1], in0=msc[:, ci:ci + 1],
                             in1=gamma_sb[:, :])

        # PT column-block for this ci: pt_sb[:, dj, ci] = P[ci, dj].T
        for dj in range(CI):
            # transpose in bf16 (fp8 PE-transpose needs strided out APs);
            # the PSUM->SBUF copy below casts to fp8 for the DoubleRow matmul.
            # Copies alternate ACT/DVE so dj pairs land concurrently and the
            # first einsum2 matmul (needs dj 0,1) unblocks sooner.
            pt_ps = opool.tile([P, P], BF16, name="pt_ps", tag="op")
            nc.tensor.transpose(
                out=pt_ps[:, :],
                in_=p_sb[:, ci, dj * P:(dj + 1) * P],
                identity=ident[:, :],
            )
            if dj % 2 == 0:
                nc.scalar.copy(out=pt_sb[:, dj, ci * P:(ci + 1) * P],
                               in_=pt_ps[:, :])
            else:
                nc.vector.tensor_copy(out=pt_sb[:, dj, ci * P:(ci + 1) * P],
                                      in_=pt_ps[:, :])

    DEFER = 3      # einsum2 column-groups deferred into the next row-block's
                   # softmax window, so the PE never idles there
    # ---- row-block 0: full softmax -> PT -> einsum2 chain, issued before
    # rows 1-3's einsum1 so every engine prioritizes the first-store chain
    with tc.high_priority():
        softmax_pt(0)
    for sj in range(SJ - DEFER):
        einsum2_group(0, sj, None)
    deferred = [(0, sj, None) for sj in range(SJ - DEFER, SJ)]

    # ---- einsum1 for rows 1-3 (mostly runs in PE idle slots during loads)
    for i, k in enumerate(KORDER):
        for ci in range(1, CI):
            nc.tensor.matmul(
                e_ps[ci][:, ci * P:],
                lhsT=vbT[:, k, ci * P:(ci + 1) * P],
                rhs=vbT[:, k, ci * P:],
                start=(i == 0),
                stop=(i == KD - 1),
            )

    for ci in range(1, CI):
        # reconstruct this row's lower blocks: e[ci, cj<ci] = e[cj, ci].T
        # (must run AFTER this row's accumulation group closes -- writing a
        # transpose into a bank with an open matmul group corrupts it)
        for cj in range(ci):
            tr_sb = otmp_pool.tile([P, P], F32, name="tr_sb", tag="tr", bufs=3)
            nc.vector.tensor_copy(out=tr_sb[:, :],
                                  in_=e_ps[cj][:, ci * P:(ci + 1) * P])
            nc.tensor.matmul(
                e_ps[ci][:, cj * P:(cj + 1) * P],
                lhsT=tr_sb[:, :],
                rhs=identf[:, :],
                is_transpose=True,
                skip_group_check=True,
            )
        for args in deferred:  # fills the PE while this ci's softmax runs
            einsum2_group(*args)
        deferred = []
        softmax_pt(ci)
        # einsum2 + epilogue for this row-block; the last DEFER column
        # groups run during the NEXT row-block's softmax window
        keep = SJ if ci == CI - 1 else SJ - DEFER
        for sj in range(keep):
            einsum2_group(ci, sj, None)
        deferred = [(ci, sj, None) for sj in range(keep, SJ)]


def build():
    nc = bacc.Bacc("TRN2", target_bir_lowering=False, debug=False,
                   num_devices=N_CORES)
    x = nc.dram_tensor("x", [C, S], F32, kind="ExternalInput")
    gamma = nc.dram_tensor("gamma", [1], F32, kind="ExternalInput")
    out = nc.dram_tensor("out", [C, S], F32, kind="ExternalOutput")
    with tile.TileContext(nc) as tc:
        with ExitStack() as ctx:
            _body(ctx, tc, out.ap(), x.ap(), gamma.ap())
    nc.compile()
    return nc


_NC_CACHE = {}
LAST_RESULTS = None


def kernel(x: np.ndarray, gamma: np.ndarray) -> np.ndarray:
    global LAST_RESULTS
    x = np.ascontiguousarray(np.asarray(x, dtype=np.float32))
    gamma = np.ascontiguousarray(np.asarray(gamma, dtype=np.float32))
    n, c, h, w = x.shape
    assert (n, c, h * w) == (N_CORES, C, S), f"unexpected shape {x.shape}"

    # NTFF tracing is unavailable through this execution path; make sure an
    # inherited BASS_TRACE=1 cannot divert run_bass_kernel_spmd into it.
    os.environ["BASS_NEVER_TRACE"] = "1"

    if "nc" not in _NC_CACHE:
        _NC_CACHE["nc"] = build()
    nc = _NC_CACHE["nc"]

    in_maps = [
        {"x": x[i].reshape(C, S), "gamma": gamma} for i in range(N_CORES)
    ]
    res = run_bass_kernel_spmd(nc, in_maps, core_ids=list(range(N_CORES)))
    LAST_RESULTS = res
    out = np.stack([res.results[i]["out"] for i in range(N_CORES)], axis=0)
    return out.reshape(n, c, h, w).astype(np.float32, copy=False)


if __name__ == "__main__":
    xs = np.random.randn(N_CORES, C, 64, 64).astype(np.float32)
    g = np.zeros((1,), np.float32)
    o = kernel(xs, g)
    print("ok", o.shape, np.abs(o - xs).max())



# revision 28
# speedup vs baseline: 1.0027x; 1.0027x over previous
"""Channel-attention (CAM) Trainium2 Bass kernel.

Reference computation (per batch n):
    v = x[n].reshape(C, S)                 # C=512, S=H*W=4096
    energy = v @ v.T                       # (C, C)
    att = softmax(max_row(energy) - energy, axis=-1)
        = exp(min_row(energy) - energy) / Z
    out[n] = gamma * (att @ v) + x[n]

Sharding: data-parallel over N=8 batches across 8 NeuronCores; each core
computes one full C x C attention locally (no collectives).

The kernel is HBM-bound: 8 MiB in + 8 MiB out at the 360 GB/s modeled DMA
rate is ~46.6 us; every energy element needs every input byte, so the
output stream cannot start until all loads land plus one softmax-pipeline
latency. The design minimizes that mid-kernel DMA bubble:

  1. Loads stream as 512-col stripes. The LAST stripe (cols 3584-4096) is
     issued from the ACT queue so it lands EARLY in the DMA order; the
     true last-arriving stripe (6) then flows through the already-warm
     bulk pipeline instead of a cold end-of-load convoy.
  2. Per stripe, DVE casts to bf16 (vb) and GpSimd to fp8-e4m3 (vb8).
  3. vT: every chunk is PE-transposed (bf16) during the load phase into
     rotating PSUM staging; PSUM->SBUF copies go 3/4 ACT / 1/4 DVE so the
     DVE queue is empty when the final stripe lands (the copy stream is
     the end-game serial bottleneck); the last four chunks alternate
     engines.
  4. einsum1 energy = sum_k vT_k.T @ vT_k (bf16, f32 PSUM): UPPER
     BLOCK-TRIANGLE only (row ci keeps cols >= ci*128); lower blocks are
     rebuilt by f32 PE transposes of closed upper blocks. Row 0 (the
     first-stored row) is issued first over all chunks in arrival order;
     rows 1-3 fill PE idle slots and may spill past loads-end.
  5. Row pipeline: DVE row-min -> ACT exp (accum_out = Z) -> PE
     transposes of P (copies alternate ACT/DVE, cast to fp8) -> einsum2
     att @ v as fp8 DoubleRow pair-matmuls -> fused epilogue
     xf = o_ps*(gamma/Z) + xf in ONE DVE scalar_tensor_tensor (first 3
     column-groups per row; later groups alternate with an ACT-scale +
     GpSimd-add path) -> store straight from xf. Two einsum2 groups per
     row are deferred into the next row''s softmax window.

Cost-model (TimelineSim): ~60.8 us/core vs ~47 us DMA-roofline; the
remaining gap is the post-load softmax-pipeline latency (~10 us bubble:
900ns DMA-completion sem, last-stripe cast/transpose/copy drain, row-0
close, reduce+exp+PT+einsum2+epilogue chain, ~1.3 us store-issue
overhead).
"""

import os
from contextlib import ExitStack

import numpy as np

import concourse.bass as bass
import concourse.tile as tile
from concourse import bacc, mybir
from concourse.bass_utils import run_bass_kernel_spmd
from concourse.masks import make_identity

N_CORES = 8
C = 512
S = 4096
P = 128
CI = C // P  # 4 c-chunks
KD = S // P  # 32 s-chunks of 128
SJW = 512
SJ = S // SJW  # 8 s-chunks of 512
TGROUPS = 1  # xbar-transpose DMA granularity (groups over the tail of S)

F32 = mybir.dt.float32
BF16 = mybir.dt.bfloat16
FP8 = mybir.dt.float8e4


def _body(ctx: ExitStack, tc: tile.TileContext, out: bass.AP, x: bass.AP,
          gamma: bass.AP):
    nc = tc.nc

    persist = ctx.enter_context(tc.tile_pool(name="persist", bufs=1))
    xf = persist.tile([P, CI, S], F32, name="xf")
    vb = persist.tile([P, CI, S], BF16, name="vb")
    vb8 = persist.tile([P, CI, S], FP8, name="vb8")
    vbT = persist.tile([P, KD, C], BF16, name="vbT")
    p_sb = persist.tile([P, CI, C], BF16, name="p_sb")
    pt_sb = persist.tile([P, CI, C], FP8, name="pt_sb")
    ident = persist.tile([P, P], BF16, name="ident")
    identf = persist.tile([P, P], F32, name="identf")
    gamma_sb = persist.tile([P, 1], F32, name="gamma_sb")
    mn = persist.tile([P, CI], F32, name="mn")
    zsum = persist.tile([P, CI], F32, name="zsum")
    msc = persist.tile([P, CI], F32, name="msc")

    x3 = x.rearrange("(ci p) s -> p ci s", p=P)
    out3 = out.rearrange("(ci p) s -> p ci s", p=P)

    # ---- loads first (earliest possible DMA start), then casts ----
    # The LAST stripe (cols 3584-4096) is issued from the ACT/DVE queues so
    # it lands EARLY in the real DMA order (their HWDGE slots race ahead of
    # the 28 sync-queued loads). The compile-time tile scheduler then emits
    # every stripe-7-dependent instruction early in each engine's in-order
    # stream, and the true last-arriving stripe (6) flows through the
    # already-warm bulk pipeline -- this removes the end-of-load convoy
    # that otherwise delays the first energy row by ~3us.
    KPE = 28                    # chunks in stripes 0-6 (stripe 6 arrives last)
    stripe_cols = [512] * 8
    tail_sl = slice(KPE * P, KD * P)
    for ci in range(CI):
        nc.scalar.dma_start(out=xf[:, ci, tail_sl], in_=x3[:, ci, tail_sl])
    col = 0
    for w in stripe_cols[:-1]:
        sl = slice(col, col + w)
        for ci in range(CI):
            nc.sync.dma_start(out=xf[:, ci, sl], in_=x3[:, ci, sl])
        col += w

    make_identity(nc, ident)
    make_identity(nc, identf)
    nc.gpsimd.dma_start(out=gamma_sb[:, :], in_=gamma.to_broadcast((P, 1)))

    # casts in arrival order: stripe 7 first (it lands first), then 0-6;
    # DVE makes the bf16 copy, GpSimd the fp8 copy for einsum2
    for ci in range(CI):
        nc.vector.tensor_copy(out=vb[:, ci, tail_sl], in_=xf[:, ci, tail_sl])
        nc.gpsimd.tensor_copy(out=vb8[:, ci, tail_sl], in_=xf[:, ci, tail_sl])
    col = 0
    for w in stripe_cols[:-1]:
        sl = slice(col, col + w)
        for ci in range(CI):
            nc.vector.tensor_copy(out=vb[:, ci, sl], in_=xf[:, ci, sl])
            nc.gpsimd.tensor_copy(out=vb8[:, ci, sl], in_=xf[:, ci, sl])
        col += w

    # ---- vT: ALL chunks transposed on the PE (it is idle during loads;
    # xbar transposes would serialize ~3.4us of HWDGE overhead right on the
    # load->softmax critical path) ----
    # PSUM->SBUF copies alternate ACT/DVE: a single-engine copy stream
    # (28 x 612ns on ACT) would be the serial bottleneck gating einsum1
    # PSUM pools are bank-granular: 4 energy banks + 4 rotating banks
    opool = ctx.enter_context(tc.tile_pool(name="opool", bufs=4, space="PSUM"))
    KORDER = list(range(KPE, KD)) + list(range(KPE))  # chunk arrival order
    from contextlib import nullcontext
    for i, k in enumerate(KORDER):
        tp_ps = opool.tile([P, C], BF16, name="tp_ps", tag="op")
        # last-arriving chunks: high priority so the PE prefers their
        # transposes over rows-1-3 einsum work the moment casts land
        hp = tc.high_priority() if i >= KD - 4 else nullcontext()
        with hp:
            for ci in range(CI):
                nc.tensor.transpose(
                    out=tp_ps[:, ci * P:(ci + 1) * P],
                    in_=vb[:, ci, k * P:(k + 1) * P],
                    identity=ident[:, :],
                )
        # ACT takes 3/4 of the copies so DVE (which also does every cast)
        # keeps an empty queue near the end of the load phase; the last
        # stripe alternates so its four copies drain on two engines
        if i >= KD - 4:
            use_dve = (i % 2 == 1)
        else:
            use_dve = (i % 4 == 1)
        if use_dve:
            nc.vector.tensor_copy(out=vbT[:, k, :], in_=tp_ps[:, :])
        else:
            nc.scalar.copy(out=vbT[:, k, :], in_=tp_ps[:, :])

    # ---- einsum1: energy = v @ v.T ----
    # Row-block 0 first over ALL chunks (it gates the first store); rows 1-3
    # follow with higher scheduler priority numbers, so the PE prefers
    # row-0-critical work whenever both are ready.
    epool = ctx.enter_context(tc.tile_pool(name="epool", bufs=4, space="PSUM"))
    e_ps = [epool.tile([P, C], F32, name=f"e{ci}", tag="et") for ci in range(CI)]
    # energy is symmetric: compute only the upper block-triangle (row ci
    # keeps columns >= ci*P); the lower blocks are reconstructed by
    # transposing the finished upper blocks on the PE (f32)
    for i, k in enumerate(KORDER):
        nc.tensor.matmul(
            e_ps[0][:, :],
            lhsT=vbT[:, k, 0:P],
            rhs=vbT[:, k, :],
            start=(i == 0),
            stop=(i == KD - 1),
        )

    otmp_pool = ctx.enter_context(tc.tile_pool(name="otmp", bufs=6))

    def einsum2_group(ci, sj, _unused):
        # fp8 DoubleRow: each matmul contracts a PAIR of 128-d chunks
        o_ps = opool.tile([P, SJW], F32, name="o_ps", tag="op")
        for h in range(CI // 2):
            nc.tensor.matmul(
                o_ps[:, :],
                lhsT=pt_sb[:, 2 * h:2 * h + 2, ci * P:(ci + 1) * P],
                rhs=vb8[:, 2 * h:2 * h + 2, sj * SJW:(sj + 1) * SJW],
                start=(h == 0),
                stop=(h == CI // 2 - 1),
                perf_mode=mybir.MatmulPerfMode.DoubleRow,
            )
        hs = slice(sj * SJW, (sj + 1) * SJW)
        # first three groups of a row all take the short DVE path so the
        # store stream ramps without a gap; afterwards alternate DVE /
        # ACT+GpSimd to balance engine load
        if sj < 3 or sj % 2 == 0:
            # fused epilogue: xf = o_ps * (gamma/Z) + xf in ONE DVE op
            nc.vector.scalar_tensor_tensor(
                out=xf[:, ci, hs], in0=o_ps[:, :], scalar=msc[:, ci:ci + 1],
                in1=xf[:, ci, hs], op0=mybir.AluOpType.mult,
                op1=mybir.AluOpType.add,
            )
        else:
            # ACT scale-copy PSUM->SBUF, then GpSimd (no PSUM access) adds x
            o_tmp = otmp_pool.tile([P, SJW], F32, name="o_tmp", tag="ot")
            nc.scalar.activation(
                out=o_tmp[:, :], in_=o_ps[:, :],
                func=mybir.ActivationFunctionType.Copy,
                scale=msc[:, ci:ci + 1], bias=0.0,
            )
            nc.gpsimd.tensor_add(out=xf[:, ci, hs], in0=o_tmp[:, :],
                                 in1=xf[:, ci, hs])
        nc.sync.dma_start(out=out3[:, ci, hs], in_=xf[:, ci, hs])

    def softmax_pt(ci):
        # softmax (reversed): P = exp(min_row(e) - e), Z = row sums
        nc.vector.tensor_reduce(
            out=mn[:, ci:ci + 1], in_=e_ps[ci][:, :],
            axis=mybir.AxisListType.X, op=mybir.AluOpType.min,
        )
        nc.scalar.activation(
            out=p_sb[:, ci, :], in_=e_ps[ci][:, :],
            func=mybir.ActivationFunctionType.Exp,
            bias=mn[:, ci:ci + 1], scale=-1.0,
            accum_out=zsum[:, ci:ci + 1],
        )
        nc.vector.reciprocal(out=msc[:, ci:ci + 1], in_=zsum[:, ci:ci + 1])
        nc.vector.tensor_mul(out=msc[:, ci:ci + 1], in0=msc[:, ci:ci + 1],
                             in1=gamma_sb[:, :])

        # PT column-block for this ci: pt_sb[:, dj, ci] = P[ci, dj].T
        for dj in range(CI):
            # transpose in bf16 (fp8 PE-transpose needs strided out APs);
            # the PSUM->SBUF copy below casts to fp8 for the DoubleRow matmul.
            # Copies alternate ACT/DVE so dj pairs land concurrently and the
            # first einsum2 matmul (needs dj 0,1) unblocks sooner.
            pt_ps = opool.tile([P, P], BF16, name="pt_ps", tag="op")
            nc.tensor.transpose(
                out=pt_ps[:, :],
                in_=p_sb[:, ci, dj * P:(dj + 1) * P],
                identity=ident[:, :],
            )
            if dj % 2 == 0:
                nc.scalar.copy(out=pt_sb[:, dj, ci * P:(ci + 1) * P],
                               in_=pt_ps[:, :])
            else:
                nc.vector.tensor_copy(out=pt_sb[:, dj, ci * P:(ci + 1) * P],
                                      in_=pt_ps[:, :])

    DEFER = 2      # einsum2 column-groups deferred into the next row-block's
                   # softmax window, so the PE never idles there
    # ---- row-block 0: full softmax -> PT -> einsum2 chain, issued before
    # rows 1-3's einsum1 so every engine prioritizes the first-store chain
    with tc.high_priority():
        softmax_pt(0)
    for sj in range(SJ - DEFER):
        einsum2_group(0, sj, None)
    deferred = [(0, sj, None) for sj in range(SJ - DEFER, SJ)]

    # ---- einsum1 for rows 1-3 (mostly runs in PE idle slots during loads)
    for i, k in enumerate(KORDER):
        for ci in range(1, CI):
            nc.tensor.matmul(
                e_ps[ci][:, ci * P:],
                lhsT=vbT[:, k, ci * P:(ci + 1) * P],
                rhs=vbT[:, k, ci * P:],
                start=(i == 0),
                stop=(i == KD - 1),
            )

    for ci in range(1, CI):
        # reconstruct this row's lower blocks: e[ci, cj<ci] = e[cj, ci].T
        # (must run AFTER this row's accumulation group closes -- writing a
        # transpose into a bank with an open matmul group corrupts it)
        for cj in range(ci):
            tr_sb = otmp_pool.tile([P, P], F32, name="tr_sb", tag="tr", bufs=3)
            nc.vector.tensor_copy(out=tr_sb[:, :],
                                  in_=e_ps[cj][:, ci * P:(ci + 1) * P])
            nc.tensor.matmul(
                e_ps[ci][:, cj * P:(cj + 1) * P],
                lhsT=tr_sb[:, :],
                rhs=identf[:, :],
                is_transpose=True,
                skip_group_check=True,
            )
        for args in deferred:  # fills the PE while this ci's softmax runs
            einsum2_group(*args)
        deferred = []
        softmax_pt(ci)
        # einsum2 + epilogue for this row-block; the last DEFER column
        # groups run during the NEXT row-block's softmax window
        keep = SJ if ci == CI - 1 else SJ - DEFER
        for sj in range(keep):
            einsum2_group(ci, sj, None)
        deferred = [(ci, sj, None) for sj in range(keep, SJ)]


def build():
    nc = bacc.Bacc("TRN2", target_bir_lowering=False, debug=False,
                   num_devices=N_CORES)
    x = nc.dram_tensor("x", [C, S], F32, kind="ExternalInput")
    gamma = nc.dram_tensor("gamma", [1], F32, kind="ExternalInput")
    out = nc.dram_tensor("out", [C, S], F32, kind="ExternalOutput")
    with tile.TileContext(nc) as tc:
        with ExitStack() as ctx:
            _body(ctx, tc, out.ap(), x.ap(), gamma.ap())
    nc.compile()
    return nc


_NC_CACHE = {}
LAST_RESULTS = None


def kernel(x: np.ndarray, gamma: np.ndarray) -> np.ndarray:
    global LAST_RESULTS
    x = np.ascontiguousarray(np.asarray(x, dtype=np.float32))
    gamma = np.ascontiguousarray(np.asarray(gamma, dtype=np.float32))
    n, c, h, w = x.shape
    assert (n, c, h * w) == (N_CORES, C, S), f"unexpected shape {x.shape}"

    # NTFF tracing is unavailable through this execution path; make sure an
    # inherited BASS_TRACE=1 cannot divert run_bass_kernel_spmd into it.
    os.environ["BASS_NEVER_TRACE"] = "1"

    if "nc" not in _NC_CACHE:
        _NC_CACHE["nc"] = build()
    nc = _NC_CACHE["nc"]

    in_maps = [
        {"x": x[i].reshape(C, S), "gamma": gamma} for i in range(N_CORES)
    ]
    res = run_bass_kernel_spmd(nc, in_maps, core_ids=list(range(N_CORES)))
    LAST_RESULTS = res
    out = np.stack([res.results[i]["out"] for i in range(N_CORES)], axis=0)
    return out.reshape(n, c, h, w).astype(np.float32, copy=False)


if __name__ == "__main__":
    xs = np.random.randn(N_CORES, C, 64, 64).astype(np.float32)
    g = np.zeros((1,), np.float32)
    o = kernel(xs, g)
    print("ok", o.shape, np.abs(o - xs).max())

